# revision 22
# baseline (speedup 1.0000x reference)
"""MoE feed-forward (top-2 of 8 routed experts + shared expert) on 8 Trainium2
NeuronCores.

Strategy: data-parallel over tokens (4096 tokens -> 512/core, no collectives).
Each core computes, for its token shard, the dense-equivalent MoE:

    out = sum_e cw[t,e] * gelu(x @ Wu[e]) @ Wd[e]  +  gelu(x @ W1) @ W2

The shared expert is folded in as a 9th expert with combine weight 1. Since
cw[t,e] = 0 for non-top-2 experts, scaling the gelu activations by cw before
the down-projection reproduces the reference exactly while letting the down
matmuls accumulate across experts.

All activations are kept transposed on chip (feature dim on partitions,
tokens on the free axis, N=512 = full shard) so both projections are plain
accumulating matmuls with naturally laid-out weights. Big matmuls run in
fp32r (full PE rate, ~1e-4 rel err); the router runs in fp32 so top-2
selection matches the fp32 reference.
"""
import os
import sys

sys.path.insert(0, "/opt/trn_rl_repo")
import numpy as np
import concourse.bass as bass
import concourse.tile as tile
from concourse import bacc, mybir
from concourse.bass_utils import run_bass_kernel_spmd
from concourse.masks import make_identity

F32 = mybir.dt.float32
F32R = mybir.dt.float32r
F16 = mybir.dt.float16
AF = mybir.ActivationFunctionType
ALU = mybir.AluOpType

B, S, D, F, E = 2, 2048, 1024, 2048, 8
T = B * S
NCORES = 8
TC = T // NCORES            # 512 tokens per core
E9 = E + 1                  # 8 routed experts + shared
KD = D // 128               # 8 contraction chunks over D
FJ = F // 128               # 16 f-chunks per expert
ORDER = [E] + list(range(E))  # shared expert first (doesn't need router output)

_CACHE = {}


def _emit(nc, tc, ctx, aps):
    xt, xt16, wu, wd, wg, selc, yt = aps
    sb = ctx.enter_context(tc.tile_pool(name="sb", bufs=1))
    ps = ctx.enter_context(tc.tile_pool(name="ps", bufs=1, space="PSUM"))

    # ---- x loads: fp16 x first (first up-group gates on it), router x after.
    # The first up-group's weights are prefetched interleaved with x16 in
    # fine 256KB pieces so the first matmuls gate on minimal bytes.
    x_r = []
    first_wts = []
    for k in range(KD):
        t_ = sb.tile([128, TC], F16, name=f"x_r{k}")
        nc.sync.dma_start(out=t_, in_=xt16[k])
        x_r.append(t_)
        w_ = sb.tile([128, TC], F16, name=f"wu_first_{k}", tag="xf", bufs=8)
        nc.sync.dma_start(out=w_, in_=wu[ORDER[0], 0, k // 4][:, k % 4, :])
        first_wts.append(w_)

    wg_sb = sb.tile([128, KD, E], F32, name="wg_sb")
    for k in range(KD):
        nc.sync.dma_start(out=wg_sb[:, k, :], in_=wg[k])
    x_f = []
    for k in range(KD):
        t_ = sb.tile([128, TC], F32, name=f"x_f{k}", tag="xf", bufs=8)
        nc.sync.dma_start(out=t_, in_=xt[k].bitcast(F32))
        x_f.append(t_)

    ident = sb.tile([128, 128], F32, name="ident")
    make_identity(nc, ident)


    # ---- router: logits -> top-2 -> combine weights, transposed to [E, TC] ----
    cwT = sb.tile([E, TC], F32R, name="cwT")
    cw_tiles = [None] * (TC // 128)

    def router_logits(tt):
        lg_ps = ps.tile([128, E], F32, name=f"lg_ps{tt}", tag="dn", bufs=4)
        for k in range(KD):
            nc.tensor.matmul(
                lg_ps,
                x_f[k][:, tt * 128:(tt + 1) * 128],
                wg_sb[:, k, :],
                start=(k == 0), stop=(k == KD - 1),
            )
        lg = sb.tile([128, E], F32, name=f"lg{tt}", tag="rsmall", bufs=8)
        nc.vector.tensor_copy(lg, lg_ps)
        v1 = sb.tile([128, 1], F32, name=f"v1_{tt}", tag="rtiny", bufs=16)
        nc.vector.reduce_max(out=v1, in_=lg, axis=mybir.AxisListType.X)
        m1 = sb.tile([128, E], F32, name=f"m1_{tt}", tag="rsmall", bufs=8)
        nc.vector.tensor_scalar(out=m1, in0=lg, scalar1=v1, scalar2=None,
                                op0=ALU.is_equal)
        # mask out the argmax, find the second max
        l2 = sb.tile([128, E], F32, name=f"l2_{tt}", tag="rsmall", bufs=8)
        big = sb.tile([128, E], F32, name=f"big_{tt}", tag="rsmall", bufs=8)
        nc.vector.tensor_scalar(out=big, in0=m1, scalar1=1e30, scalar2=None,
                                op0=ALU.mult)
        nc.vector.tensor_sub(out=l2, in0=lg, in1=big)
        v2 = sb.tile([128, 1], F32, name=f"v2_{tt}", tag="rtiny", bufs=16)
        nc.vector.reduce_max(out=v2, in_=l2, axis=mybir.AxisListType.X)
        m2 = sb.tile([128, E], F32, name=f"m2_{tt}", tag="rsmall", bufs=8)
        nc.vector.tensor_scalar(out=m2, in0=l2, scalar1=v2, scalar2=None,
                                op0=ALU.is_equal)
        # renormalized top-2 weights: w1 = sigmoid(l1 - l2), w2 = 1 - w1
        d12 = sb.tile([128, 1], F32, name=f"d12_{tt}", tag="rtiny", bufs=16)
        nc.vector.tensor_sub(out=d12, in0=v1, in1=v2)
        w1 = sb.tile([128, 1], F32, name=f"w1_{tt}", tag="rtiny", bufs=16)
        nc.scalar.activation(out=w1, in_=d12, func=AF.Sigmoid)
        w2 = sb.tile([128, 1], F32, name=f"w2_{tt}", tag="rtiny", bufs=16)
        nc.vector.tensor_scalar(out=w2, in0=w1, scalar1=-1.0, scalar2=-1.0,
                                op0=ALU.mult, op1=ALU.subtract)
        cw1 = sb.tile([128, E], F32, name=f"cw1_{tt}", tag="rsmall", bufs=8)
        nc.vector.tensor_scalar(out=cw1, in0=m1, scalar1=w1, scalar2=None,
                                op0=ALU.mult)
        cw2 = sb.tile([128, E], F32, name=f"cw2_{tt}", tag="rsmall", bufs=8)
        nc.vector.tensor_scalar(out=cw2, in0=m2, scalar1=w2, scalar2=None,
                                op0=ALU.mult)
        cw = sb.tile([128, E], F32, name=f"cw_{tt}", tag="rsmall", bufs=8)
        nc.vector.tensor_add(out=cw, in0=cw1, in1=cw2)
        cw_tiles[tt] = cw

    def router_transpose(tt):
        # transpose [128 tok, E] -> [E, 128 tok] into the cwT column block
        tr_ps = ps.tile([E, 128], F32, name=f"tr_ps{tt}", tag="dn", bufs=4)
        nc.tensor.transpose(tr_ps, cw_tiles[tt], ident)
        nc.vector.tensor_copy(cwT[:, tt * 128:(tt + 1) * 128], tr_ps)

    # ---- h tiles (per expert slot x f-chunk), acc tiles ----
    def up_phase(e, extras=None, prefetched=None):
        """hT(e) = gelu(Wu[e].T-chunks @ x), scaled by cw row e (routed only)."""
        h_tiles = []
        for jj in range(4):
            if extras and jj in extras:
                extras[jj]()            # groups of 4 f-chunks / 4 psum banks
            if jj == 0 and prefetched:
                wts = None
            else:
                wts = []
                for kk in range(KD // 4):
                    w_ = sb.tile([128, 4, 512], F16, name=f"wu_{e}_{jj}_{kk}",
                                 tag="wu", bufs=10)
                    nc.sync.dma_start(out=w_, in_=wu[e, jj, kk])
                    wts.append(w_)
            grp = [ps.tile([128, TC], F32, name=f"up_ps_{e}_{jj}_{j2}",
                           tag="up", bufs=4) for j2 in range(4)]
            for k in range(KD):
                for j2 in range(4):
                    lhsT = (prefetched[k][:, j2 * 128:(j2 + 1) * 128]
                            if wts is None else
                            wts[k // 4][:, k % 4, j2 * 128:(j2 + 1) * 128])
                    nc.tensor.matmul(
                        grp[j2],
                        lhsT,
                        x_r[k],
                        start=(k == 0), stop=(k == KD - 1),
                    )
            for j2 in range(4):
                j = jj * 4 + j2
                h_ = sb.tile([128, TC], F16, name=f"h_{e}_{j}",
                             tag=f"h{j}", bufs=2)
                nc.scalar.activation(out=h_, in_=grp[j2], func=AF.Gelu)
                if e != E:
                    nc.vector.tensor_mul(out=h_, in0=h_, in1=rep[e])
                h_tiles.append(h_)
        return h_tiles

    def down_phase(e, h_tiles, first, fine=False):
        """acc += Wd[e].T-chunks @ hT'(e), accumulated over f in PSUM.

        fine=True (last expert): 2-bank passes so the tail's accumulate +
        output DMA overlap the remaining matmuls."""
        nbank = 2 if fine else 4
        for ii in range(2):            # d_out halves
            wts = []
            for jh in range(FJ // 4):
                w_ = sb.tile([128, 4, 512], F16, name=f"wd_{e}_{jh}_{ii}",
                             tag="wd", bufs=16)
                nc.sync.dma_start(out=w_, in_=wd[e, jh, ii])
                wts.append(w_)
            for sub in range(4 // nbank):
                grp = [ps.tile([128, TC], F32, name=f"dn_ps_{e}_{ii}_{sub}_{i2}",
                               tag="dn", bufs=4) for i2 in range(nbank)]
                for j in range(FJ):
                    for i2 in range(nbank):
                        ic = sub * nbank + i2
                        nc.tensor.matmul(
                            grp[i2],
                            wts[j // 4][:, j % 4, ic * 128:(ic + 1) * 128],
                            h_tiles[j],
                            start=(j == 0), stop=(j == FJ - 1),
                        )
                for i2 in range(nbank):
                    i = ii * 4 + sub * nbank + i2
                    if first:
                        nc.vector.tensor_copy(acc[i], grp[i2])
                    else:
                        nc.vector.tensor_add(out=acc[i], in0=acc[i], in1=grp[i2])
                    if fine:
                        nc.sync.dma_start(out=yt[i], in_=acc[i])

    acc = [sb.tile([128, TC], F32, name=f"acc{i}") for i in range(KD)]
    rep = [None] * E

    # shared expert's up phase first: it only needs x16 + its weights, so the
    # PE starts ~10us earlier than if the router (which gates on all 8 fp32
    # x chunks) came first. The router runs right after, well before rep[] is
    # needed by up(0)'s scale.
    def _lg_all():
        for tt in range(TC // 128):
            router_logits(tt)

    def _tr_all():
        for tt in range(TC // 128):
            router_transpose(tt)

    # combine-weight rows broadcast across partitions via selector matmuls:
    # rep[e][p, t] = cw[t, e] for all p
    def selectors():
        for e in range(E):
            sel = sb.tile([E, 128], F32R, name=f"sel{e}", tag="sel", bufs=2)
            nc.sync.dma_start(out=sel, in_=selc[e])
            r_ps = ps.tile([128, TC], F32, name=f"rep_ps{e}", tag="dn", bufs=4)
            nc.tensor.matmul(r_ps, sel, cwT, start=True, stop=True)
            r_ = sb.tile([128, TC], F32, name=f"rep{e}")
            nc.scalar.copy(r_, r_ps)
            rep[e] = r_

    h_cur = up_phase(ORDER[0], extras={1: _lg_all, 2: _tr_all, 3: selectors},
                     prefetched=first_wts)

    # software-pipelined main loop: up(e_next) is emitted before down(e) so the
    # PE always has independent matmul work while gelu/scale of e_next runs.
    for idx in range(1, E9):
        h_next = up_phase(ORDER[idx])
        down_phase(ORDER[idx - 1], h_cur, first=(idx == 1))
        h_cur = h_next
    down_phase(ORDER[E9 - 1], h_cur, first=False, fine=True)


def _build():
    if "nc" in _CACHE:
        return _CACHE["nc"]
    nc = bacc.Bacc("TRN2", target_bir_lowering=False, debug=False)
    xt = nc.dram_tensor("xt", [KD, 128, TC], F32R, kind="ExternalInput").ap()
    xt16 = nc.dram_tensor("xt16", [KD, 128, TC], F16, kind="ExternalInput").ap()
    wu = nc.dram_tensor("wu", [E9, 4, KD // 4, 128, 4, 512], F16, kind="ExternalInput").ap()
    wd = nc.dram_tensor("wd", [E9, FJ // 4, 2, 128, 4, 512], F16, kind="ExternalInput").ap()
    wg = nc.dram_tensor("wg", [KD, 128, E], F32, kind="ExternalInput").ap()
    selc = nc.dram_tensor("selc", [E, E, 128], F32R, kind="ExternalInput").ap()
    yt = nc.dram_tensor("yt", [KD, 128, TC], F32, kind="ExternalOutput").ap()
    from contextlib import ExitStack
    with tile.TileContext(nc) as tc, ExitStack() as ctx:
        _emit(nc, tc, ctx, (xt, xt16, wu, wd, wg, selc, yt))
    nc.compile()
    _CACHE["nc"] = nc
    return nc


def kernel(x, Wg, Wu, Wd, W1, W2):
    x = np.ascontiguousarray(x, dtype=np.float32)
    Wg = np.ascontiguousarray(Wg, dtype=np.float32)
    Wu_all = np.concatenate([Wu, W1[None]], axis=0).astype(np.float32)   # [9,D,F]
    Wd_all = np.concatenate([Wd, W2[None]], axis=0).astype(np.float32)   # [9,F,D]

    # host-side tiling to DMA-friendly layouts
    # wu_t[e, jj, kk, p, h, ff] = Wu_all[e, (2*kk+h)*128 + p, jj*512 + ff]
    wu_t = np.ascontiguousarray(
        Wu_all.reshape(E9, KD // 4, 4, 128, 4, 512).transpose(0, 4, 1, 3, 2, 5)
        .astype(np.float16))
    # wd_t[e, jh, ii, p, g, dd] = Wd_all[e, (2*jh+g)*128 + p, ii*512 + dd]
    wd_t = np.ascontiguousarray(
        Wd_all.reshape(E9, FJ // 4, 4, 128, 2, 512).transpose(0, 1, 4, 3, 2, 5)
        .astype(np.float16))
    wg_t = np.ascontiguousarray(Wg.reshape(KD, 128, E))
    sel_c = np.zeros((E, E, 128), dtype=np.float32)
    for e in range(E):
        sel_c[e, e, :] = 1.0

    x2 = x.reshape(T, D)
    in_maps = []
    for c in range(NCORES):
        xt_c = np.ascontiguousarray(
            x2[c * TC:(c + 1) * TC].T).reshape(KD, 128, TC)
        in_maps.append({"xt": xt_c, "xt16": xt_c.astype(np.float16),
                        "wu": wu_t, "wd": wd_t, "wg": wg_t, "selc": sel_c})

    nc = _build()
    trace = bool(os.environ.get("MOE_TRACE"))
    res = run_bass_kernel_spmd(
        nc, in_maps, list(range(NCORES)),
        trace=trace, trace_cores=list(range(NCORES)) if trace else None,
    )
    _CACHE["last_result"] = res

    out = np.empty((T, D), dtype=np.float32)
    for c in range(NCORES):
        out[c * TC:(c + 1) * TC] = res.results[c]["yt"].reshape(D, TC).T
    return out.reshape(B, S, D)


# revision 23
# speedup vs baseline: 1.0010x; 1.0010x over previous
"""MoE feed-forward (top-2 of 8 routed experts + shared expert) on 8 Trainium2
NeuronCores.

Strategy: data-parallel over tokens (4096 tokens -> 512/core, no collectives).
Each core computes, for its token shard, the dense-equivalent MoE:

    out = sum_e cw[t,e] * gelu(x @ Wu[e]) @ Wd[e]  +  gelu(x @ W1) @ W2

The shared expert is folded in as a 9th expert with combine weight 1. Since
cw[t,e] = 0 for non-top-2 experts, scaling the gelu activations by cw before
the down-projection reproduces the reference exactly while letting the down
matmuls accumulate across experts.

All activations are kept transposed on chip (feature dim on partitions,
tokens on the free axis, N=512 = full shard) so both projections are plain
accumulating matmuls with naturally laid-out weights. Big matmuls run in
fp32r (full PE rate, ~1e-4 rel err); the router runs in fp32 so top-2
selection matches the fp32 reference.
"""
import os
import sys

sys.path.insert(0, "/opt/trn_rl_repo")
import numpy as np
import concourse.bass as bass
import concourse.tile as tile
from concourse import bacc, mybir
from concourse.bass_utils import run_bass_kernel_spmd
from concourse.masks import make_identity

F32 = mybir.dt.float32
F32R = mybir.dt.float32r
F16 = mybir.dt.float16
AF = mybir.ActivationFunctionType
ALU = mybir.AluOpType

B, S, D, F, E = 2, 2048, 1024, 2048, 8
T = B * S
NCORES = 8
TC = T // NCORES            # 512 tokens per core
E9 = E + 1                  # 8 routed experts + shared
KD = D // 128               # 8 contraction chunks over D
FJ = F // 128               # 16 f-chunks per expert
ORDER = [E] + list(range(E))  # shared expert first (doesn't need router output)

_CACHE = {}


def _emit(nc, tc, ctx, aps):
    xt, xt16, wu, wd, wg, selc, yt = aps
    sb = ctx.enter_context(tc.tile_pool(name="sb", bufs=1))
    ps = ctx.enter_context(tc.tile_pool(name="ps", bufs=1, space="PSUM"))

    # ---- x loads: fp16 x first (first up-group gates on it), router x after.
    # The first up-group's weights are prefetched interleaved with x16 in
    # fine 256KB pieces so the first matmuls gate on minimal bytes.
    x_r = []
    first_wts = []
    for k in range(KD):
        t_ = sb.tile([128, TC], F16, name=f"x_r{k}")
        nc.sync.dma_start(out=t_, in_=xt16[k])
        x_r.append(t_)
        w_ = sb.tile([128, TC], F16, name=f"wu_first_{k}", tag="wuf", bufs=8)
        nc.sync.dma_start(out=w_, in_=wu[ORDER[0], 0, k // 4][:, k % 4, :])
        first_wts.append(w_)

    wg_sb = sb.tile([128, KD, E], F32, name="wg_sb")
    for k in range(KD):
        nc.sync.dma_start(out=wg_sb[:, k, :], in_=wg[k])
    x_f = []
    for k in range(KD):
        t_ = sb.tile([128, TC], F32, name=f"x_f{k}", tag="xf", bufs=8)
        nc.sync.dma_start(out=t_, in_=xt[k].bitcast(F32))
        x_f.append(t_)

    ident = sb.tile([128, 128], F32, name="ident")
    make_identity(nc, ident)


    # ---- router: logits -> top-2 -> combine weights, transposed to [E, TC] ----
    cwT = sb.tile([E, TC], F32R, name="cwT")
    cw_tiles = [None] * (TC // 128)

    def router_logits(tt):
        lg_ps = ps.tile([128, E], F32, name=f"lg_ps{tt}", tag="dn", bufs=4)
        for k in range(KD):
            nc.tensor.matmul(
                lg_ps,
                x_f[k][:, tt * 128:(tt + 1) * 128],
                wg_sb[:, k, :],
                start=(k == 0), stop=(k == KD - 1),
            )
        lg = sb.tile([128, E], F32, name=f"lg{tt}", tag="rsmall", bufs=8)
        nc.vector.tensor_copy(lg, lg_ps)
        v1 = sb.tile([128, 1], F32, name=f"v1_{tt}", tag="rtiny", bufs=16)
        nc.vector.reduce_max(out=v1, in_=lg, axis=mybir.AxisListType.X)
        m1 = sb.tile([128, E], F32, name=f"m1_{tt}", tag="rsmall", bufs=8)
        nc.vector.tensor_scalar(out=m1, in0=lg, scalar1=v1, scalar2=None,
                                op0=ALU.is_equal)
        # mask out the argmax, find the second max
        l2 = sb.tile([128, E], F32, name=f"l2_{tt}", tag="rsmall", bufs=8)
        big = sb.tile([128, E], F32, name=f"big_{tt}", tag="rsmall", bufs=8)
        nc.vector.tensor_scalar(out=big, in0=m1, scalar1=1e30, scalar2=None,
                                op0=ALU.mult)
        nc.vector.tensor_sub(out=l2, in0=lg, in1=big)
        v2 = sb.tile([128, 1], F32, name=f"v2_{tt}", tag="rtiny", bufs=16)
        nc.vector.reduce_max(out=v2, in_=l2, axis=mybir.AxisListType.X)
        m2 = sb.tile([128, E], F32, name=f"m2_{tt}", tag="rsmall", bufs=8)
        nc.vector.tensor_scalar(out=m2, in0=l2, scalar1=v2, scalar2=None,
                                op0=ALU.is_equal)
        # renormalized top-2 weights: w1 = sigmoid(l1 - l2), w2 = 1 - w1
        d12 = sb.tile([128, 1], F32, name=f"d12_{tt}", tag="rtiny", bufs=16)
        nc.vector.tensor_sub(out=d12, in0=v1, in1=v2)
        w1 = sb.tile([128, 1], F32, name=f"w1_{tt}", tag="rtiny", bufs=16)
        nc.scalar.activation(out=w1, in_=d12, func=AF.Sigmoid)
        w2 = sb.tile([128, 1], F32, name=f"w2_{tt}", tag="rtiny", bufs=16)
        nc.vector.tensor_scalar(out=w2, in0=w1, scalar1=-1.0, scalar2=-1.0,
                                op0=ALU.mult, op1=ALU.subtract)
        cw1 = sb.tile([128, E], F32, name=f"cw1_{tt}", tag="rsmall", bufs=8)
        nc.vector.tensor_scalar(out=cw1, in0=m1, scalar1=w1, scalar2=None,
                                op0=ALU.mult)
        cw2 = sb.tile([128, E], F32, name=f"cw2_{tt}", tag="rsmall", bufs=8)
        nc.vector.tensor_scalar(out=cw2, in0=m2, scalar1=w2, scalar2=None,
                                op0=ALU.mult)
        cw = sb.tile([128, E], F32, name=f"cw_{tt}", tag="rsmall", bufs=8)
        nc.vector.tensor_add(out=cw, in0=cw1, in1=cw2)
        cw_tiles[tt] = cw

    def router_transpose(tt):
        # transpose [128 tok, E] -> [E, 128 tok] into the cwT column block
        tr_ps = ps.tile([E, 128], F32, name=f"tr_ps{tt}", tag="dn", bufs=4)
        nc.tensor.transpose(tr_ps, cw_tiles[tt], ident)
        nc.vector.tensor_copy(cwT[:, tt * 128:(tt + 1) * 128], tr_ps)

    # ---- h tiles (per expert slot x f-chunk), acc tiles ----
    def up_phase(e, extras=None, prefetched=None):
        """hT(e) = gelu(Wu[e].T-chunks @ x), scaled by cw row e (routed only)."""
        h_tiles = []
        for jj in range(4):
            if extras and jj in extras:
                extras[jj]()            # groups of 4 f-chunks / 4 psum banks
            if jj == 0 and prefetched:
                wts = None
            else:
                wts = []
                for kk in range(KD // 4):
                    w_ = sb.tile([128, 4, 512], F16, name=f"wu_{e}_{jj}_{kk}",
                                 tag="wu", bufs=10)
                    nc.sync.dma_start(out=w_, in_=wu[e, jj, kk])
                    wts.append(w_)
            grp = [ps.tile([128, TC], F32, name=f"up_ps_{e}_{jj}_{j2}",
                           tag="up", bufs=4) for j2 in range(4)]
            for k in range(KD):
                for j2 in range(4):
                    lhsT = (prefetched[k][:, j2 * 128:(j2 + 1) * 128]
                            if wts is None else
                            wts[k // 4][:, k % 4, j2 * 128:(j2 + 1) * 128])
                    nc.tensor.matmul(
                        grp[j2],
                        lhsT,
                        x_r[k],
                        start=(k == 0), stop=(k == KD - 1),
                    )
            for j2 in range(4):
                j = jj * 4 + j2
                h_ = sb.tile([128, TC], F16, name=f"h_{e}_{j}",
                             tag=f"h{j}", bufs=2)
                nc.scalar.activation(out=h_, in_=grp[j2], func=AF.Gelu)
                if e != E:
                    nc.vector.tensor_mul(out=h_, in0=h_, in1=rep[e])
                h_tiles.append(h_)
        return h_tiles

    def down_phase(e, h_tiles, first, fine=False):
        """acc += Wd[e].T-chunks @ hT'(e), accumulated over f in PSUM.

        fine=True (last expert): 2-bank passes so the tail's accumulate +
        output DMA overlap the remaining matmuls."""
        nbank = 2 if fine else 4
        for ii in range(2):            # d_out halves
            wts = []
            for jh in range(FJ // 4):
                w_ = sb.tile([128, 4, 512], F16, name=f"wd_{e}_{jh}_{ii}",
                             tag="wd", bufs=16)
                nc.sync.dma_start(out=w_, in_=wd[e, jh, ii])
                wts.append(w_)
            for sub in range(4 // nbank):
                grp = [ps.tile([128, TC], F32, name=f"dn_ps_{e}_{ii}_{sub}_{i2}",
                               tag="dn", bufs=4) for i2 in range(nbank)]
                for j in range(FJ):
                    for i2 in range(nbank):
                        ic = sub * nbank + i2
                        nc.tensor.matmul(
                            grp[i2],
                            wts[j // 4][:, j % 4, ic * 128:(ic + 1) * 128],
                            h_tiles[j],
                            start=(j == 0), stop=(j == FJ - 1),
                        )
                for i2 in range(nbank):
                    i = ii * 4 + sub * nbank + i2
                    if first:
                        nc.vector.tensor_copy(acc[i], grp[i2])
                    else:
                        nc.vector.tensor_add(out=acc[i], in0=acc[i], in1=grp[i2])
                    if fine:
                        nc.sync.dma_start(out=yt[i], in_=acc[i])

    acc = [sb.tile([128, TC], F32, name=f"acc{i}") for i in range(KD)]
    rep = [None] * E

    # shared expert's up phase first: it only needs x16 + its weights, so the
    # PE starts ~10us earlier than if the router (which gates on all 8 fp32
    # x chunks) came first. The router runs right after, well before rep[] is
    # needed by up(0)'s scale.
    def _lg_all():
        for tt in range(TC // 128):
            router_logits(tt)

    def _tr_all():
        for tt in range(TC // 128):
            router_transpose(tt)

    # combine-weight rows broadcast across partitions via selector matmuls:
    # rep[e][p, t] = cw[t, e] for all p
    def selectors():
        for e in range(E):
            sel = sb.tile([E, 128], F32R, name=f"sel{e}", tag="sel", bufs=2)
            nc.sync.dma_start(out=sel, in_=selc[e])
            r_ps = ps.tile([128, TC], F32, name=f"rep_ps{e}", tag="dn", bufs=4)
            nc.tensor.matmul(r_ps, sel, cwT, start=True, stop=True)
            r_ = sb.tile([128, TC], F32, name=f"rep{e}")
            nc.scalar.copy(r_, r_ps)
            rep[e] = r_

    h_cur = up_phase(ORDER[0], extras={1: _lg_all, 2: _tr_all, 3: selectors},
                     prefetched=first_wts)

    # software-pipelined main loop: up(e_next) is emitted before down(e) so the
    # PE always has independent matmul work while gelu/scale of e_next runs.
    for idx in range(1, E9):
        h_next = up_phase(ORDER[idx])
        down_phase(ORDER[idx - 1], h_cur, first=(idx == 1))
        h_cur = h_next
    down_phase(ORDER[E9 - 1], h_cur, first=False, fine=True)


def _build():
    if "nc" in _CACHE:
        return _CACHE["nc"]
    nc = bacc.Bacc("TRN2", target_bir_lowering=False, debug=False)
    xt = nc.dram_tensor("xt", [KD, 128, TC], F32R, kind="ExternalInput").ap()
    xt16 = nc.dram_tensor("xt16", [KD, 128, TC], F16, kind="ExternalInput").ap()
    wu = nc.dram_tensor("wu", [E9, 4, KD // 4, 128, 4, 512], F16, kind="ExternalInput").ap()
    wd = nc.dram_tensor("wd", [E9, FJ // 4, 2, 128, 4, 512], F16, kind="ExternalInput").ap()
    wg = nc.dram_tensor("wg", [KD, 128, E], F32, kind="ExternalInput").ap()
    selc = nc.dram_tensor("selc", [E, E, 128], F32R, kind="ExternalInput").ap()
    yt = nc.dram_tensor("yt", [KD, 128, TC], F32, kind="ExternalOutput").ap()
    from contextlib import ExitStack
    with tile.TileContext(nc) as tc, ExitStack() as ctx:
        _emit(nc, tc, ctx, (xt, xt16, wu, wd, wg, selc, yt))
    nc.compile()
    _CACHE["nc"] = nc
    return nc


def kernel(x, Wg, Wu, Wd, W1, W2):
    x = np.ascontiguousarray(x, dtype=np.float32)
    Wg = np.ascontiguousarray(Wg, dtype=np.float32)
    Wu_all = np.concatenate([Wu, W1[None]], axis=0).astype(np.float32)   # [9,D,F]
    Wd_all = np.concatenate([Wd, W2[None]], axis=0).astype(np.float32)   # [9,F,D]

    # host-side tiling to DMA-friendly layouts
    # wu_t[e, jj, kk, p, h, ff] = Wu_all[e, (2*kk+h)*128 + p, jj*512 + ff]
    wu_t = np.ascontiguousarray(
        Wu_all.reshape(E9, KD // 4, 4, 128, 4, 512).transpose(0, 4, 1, 3, 2, 5)
        .astype(np.float16))
    # wd_t[e, jh, ii, p, g, dd] = Wd_all[e, (2*jh+g)*128 + p, ii*512 + dd]
    wd_t = np.ascontiguousarray(
        Wd_all.reshape(E9, FJ // 4, 4, 128, 2, 512).transpose(0, 1, 4, 3, 2, 5)
        .astype(np.float16))
    wg_t = np.ascontiguousarray(Wg.reshape(KD, 128, E))
    sel_c = np.zeros((E, E, 128), dtype=np.float32)
    for e in range(E):
        sel_c[e, e, :] = 1.0

    x2 = x.reshape(T, D)
    in_maps = []
    for c in range(NCORES):
        xt_c = np.ascontiguousarray(
            x2[c * TC:(c + 1) * TC].T).reshape(KD, 128, TC)
        in_maps.append({"xt": xt_c, "xt16": xt_c.astype(np.float16),
                        "wu": wu_t, "wd": wd_t, "wg": wg_t, "selc": sel_c})

    nc = _build()
    trace = bool(os.environ.get("MOE_TRACE"))
    res = run_bass_kernel_spmd(
        nc, in_maps, list(range(NCORES)),
        trace=trace, trace_cores=list(range(NCORES)) if trace else None,
    )
    _CACHE["last_result"] = res

    out = np.empty((T, D), dtype=np.float32)
    for c in range(NCORES):
        out[c * TC:(c + 1) * TC] = res.results[c]["yt"].reshape(D, TC).T
    return out.reshape(B, S, D)


# revision 24
# speedup vs baseline: 1.0115x; 1.0105x over previous
"""MoE feed-forward (top-2 of 8 routed experts + shared expert) on 8 Trainium2
NeuronCores.

Strategy: data-parallel over tokens (4096 tokens -> 512/core, no collectives).
Each core computes, for its token shard, the dense-equivalent MoE:

    out = sum_e cw[t,e] * gelu(x @ Wu[e]) @ Wd[e]  +  gelu(x @ W1) @ W2

The shared expert is folded in as a 9th expert with combine weight 1. Since
cw[t,e] = 0 for non-top-2 experts, scaling the gelu activations by cw before
the down-projection reproduces the reference exactly while letting the down
matmuls accumulate across experts.

All activations are kept transposed on chip (feature dim on partitions,
tokens on the free axis, N=512 = full shard) so both projections are plain
accumulating matmuls with naturally laid-out weights. Big matmuls run in
fp32r (full PE rate, ~1e-4 rel err); the router runs in fp32 so top-2
selection matches the fp32 reference.
"""
import os
import sys

sys.path.insert(0, "/opt/trn_rl_repo")
import numpy as np
import concourse.bass as bass
import concourse.tile as tile
from concourse import bacc, mybir
from concourse.bass_utils import run_bass_kernel_spmd
from concourse.masks import make_identity

F32 = mybir.dt.float32
F32R = mybir.dt.float32r
F16 = mybir.dt.float16
AF = mybir.ActivationFunctionType
ALU = mybir.AluOpType

B, S, D, F, E = 2, 2048, 1024, 2048, 8
T = B * S
NCORES = 8
TC = T // NCORES            # 512 tokens per core
E9 = E + 1                  # 8 routed experts + shared
KD = D // 128               # 8 contraction chunks over D
FJ = F // 128               # 16 f-chunks per expert
ORDER = [E] + list(range(E))  # shared expert first (doesn't need router output)

_CACHE = {}


def _emit(nc, tc, ctx, aps):
    xt, xt16, wu, wd, wg, selc, yt = aps
    sb = ctx.enter_context(tc.tile_pool(name="sb", bufs=1))
    ps = ctx.enter_context(tc.tile_pool(name="ps", bufs=1, space="PSUM"))

    # ---- x loads: fp16 x first (first up-group gates on it), router x after
    x_r = []
    for k in range(KD):
        t_ = sb.tile([128, TC], F16, name=f"x_r{k}")
        nc.sync.dma_start(out=t_, in_=xt16[k])
        x_r.append(t_)
    # prefetch the first up-group's weights ahead of the router's x loads so
    # the PE's first matmul isn't queued behind 2MB of router-only data
    first_wts = []
    for k in range(KD):
        w_ = sb.tile([128, TC], F16, name=f"wu_first_{k}", tag="wuf", bufs=8)
        nc.sync.dma_start(out=w_, in_=wu[ORDER[0], 0, k // 4][:, k % 4, :])
        first_wts.append(w_)

    wg_sb = sb.tile([128, KD, E], F32, name="wg_sb")
    for k in range(KD):
        nc.sync.dma_start(out=wg_sb[:, k, :], in_=wg[k])
    x_f = []
    for k in range(KD):
        t_ = sb.tile([128, TC], F32, name=f"x_f{k}", tag="xf", bufs=8)
        nc.sync.dma_start(out=t_, in_=xt[k].bitcast(F32))
        x_f.append(t_)

    ident = sb.tile([128, 128], F32, name="ident")
    make_identity(nc, ident)


    # ---- router: logits -> top-2 -> combine weights, transposed to [E, TC] ----
    cwT = sb.tile([E, TC], F32R, name="cwT")
    cw_tiles = [None] * (TC // 128)

    def router_logits(tt):
        lg_ps = ps.tile([128, E], F32, name=f"lg_ps{tt}", tag="dn", bufs=4)
        for k in range(KD):
            nc.tensor.matmul(
                lg_ps,
                x_f[k][:, tt * 128:(tt + 1) * 128],
                wg_sb[:, k, :],
                start=(k == 0), stop=(k == KD - 1),
            )
        lg = sb.tile([128, E], F32, name=f"lg{tt}", tag="rsmall", bufs=8)
        nc.vector.tensor_copy(lg, lg_ps)
        v1 = sb.tile([128, 1], F32, name=f"v1_{tt}", tag="rtiny", bufs=16)
        nc.vector.reduce_max(out=v1, in_=lg, axis=mybir.AxisListType.X)
        m1 = sb.tile([128, E], F32, name=f"m1_{tt}", tag="rsmall", bufs=8)
        nc.vector.tensor_scalar(out=m1, in0=lg, scalar1=v1, scalar2=None,
                                op0=ALU.is_equal)
        # mask out the argmax, find the second max
        l2 = sb.tile([128, E], F32, name=f"l2_{tt}", tag="rsmall", bufs=8)
        big = sb.tile([128, E], F32, name=f"big_{tt}", tag="rsmall", bufs=8)
        nc.vector.tensor_scalar(out=big, in0=m1, scalar1=1e30, scalar2=None,
                                op0=ALU.mult)
        nc.vector.tensor_sub(out=l2, in0=lg, in1=big)
        v2 = sb.tile([128, 1], F32, name=f"v2_{tt}", tag="rtiny", bufs=16)
        nc.vector.reduce_max(out=v2, in_=l2, axis=mybir.AxisListType.X)
        m2 = sb.tile([128, E], F32, name=f"m2_{tt}", tag="rsmall", bufs=8)
        nc.vector.tensor_scalar(out=m2, in0=l2, scalar1=v2, scalar2=None,
                                op0=ALU.is_equal)
        # renormalized top-2 weights: w1 = sigmoid(l1 - l2), w2 = 1 - w1
        d12 = sb.tile([128, 1], F32, name=f"d12_{tt}", tag="rtiny", bufs=16)
        nc.vector.tensor_sub(out=d12, in0=v1, in1=v2)
        w1 = sb.tile([128, 1], F32, name=f"w1_{tt}", tag="rtiny", bufs=16)
        nc.scalar.activation(out=w1, in_=d12, func=AF.Sigmoid)
        w2 = sb.tile([128, 1], F32, name=f"w2_{tt}", tag="rtiny", bufs=16)
        nc.vector.tensor_scalar(out=w2, in0=w1, scalar1=-1.0, scalar2=-1.0,
                                op0=ALU.mult, op1=ALU.subtract)
        cw1 = sb.tile([128, E], F32, name=f"cw1_{tt}", tag="rsmall", bufs=8)
        nc.vector.tensor_scalar(out=cw1, in0=m1, scalar1=w1, scalar2=None,
                                op0=ALU.mult)
        cw2 = sb.tile([128, E], F32, name=f"cw2_{tt}", tag="rsmall", bufs=8)
        nc.vector.tensor_scalar(out=cw2, in0=m2, scalar1=w2, scalar2=None,
                                op0=ALU.mult)
        cw = sb.tile([128, E], F32, name=f"cw_{tt}", tag="rsmall", bufs=8)
        nc.vector.tensor_add(out=cw, in0=cw1, in1=cw2)
        cw_tiles[tt] = cw

    def router_transpose(tt):
        # transpose [128 tok, E] -> [E, 128 tok] into the cwT column block
        tr_ps = ps.tile([E, 128], F32, name=f"tr_ps{tt}", tag="dn", bufs=4)
        nc.tensor.transpose(tr_ps, cw_tiles[tt], ident)
        nc.vector.tensor_copy(cwT[:, tt * 128:(tt + 1) * 128], tr_ps)

    # ---- h tiles (per expert slot x f-chunk), acc tiles ----
    def up_phase(e, extras=None, prefetched=None):
        """hT(e) = gelu(Wu[e].T-chunks @ x), scaled by cw row e (routed only)."""
        h_tiles = []
        for jj in range(4):
            if extras and jj in extras:
                extras[jj]()            # groups of 4 f-chunks / 4 psum banks
            if jj == 0 and prefetched:
                wts = None
            else:
                wts = []
                for kk in range(KD // 4):
                    w_ = sb.tile([128, 4, 512], F16, name=f"wu_{e}_{jj}_{kk}",
                                 tag="wu", bufs=10)
                    nc.sync.dma_start(out=w_, in_=wu[e, jj, kk])
                    wts.append(w_)
            grp = [ps.tile([128, TC], F32, name=f"up_ps_{e}_{jj}_{j2}",
                           tag="up", bufs=4) for j2 in range(4)]
            for k in range(KD):
                for j2 in range(4):
                    lhsT = (prefetched[k][:, j2 * 128:(j2 + 1) * 128]
                            if wts is None else
                            wts[k // 4][:, k % 4, j2 * 128:(j2 + 1) * 128])
                    nc.tensor.matmul(
                        grp[j2],
                        lhsT,
                        x_r[k],
                        start=(k == 0), stop=(k == KD - 1),
                    )
            for j2 in range(4):
                j = jj * 4 + j2
                h_ = sb.tile([128, TC], F16, name=f"h_{e}_{j}",
                             tag=f"h{j}", bufs=2)
                nc.scalar.activation(out=h_, in_=grp[j2], func=AF.Gelu)
                if e != E:
                    nc.vector.tensor_mul(out=h_, in0=h_, in1=rep[e])
                h_tiles.append(h_)
        return h_tiles

    def down_phase(e, h_tiles, first, fine=False):
        """acc += Wd[e].T-chunks @ hT'(e), accumulated over f in PSUM.

        fine=True (last expert): 2-bank passes so the tail's accumulate +
        output DMA overlap the remaining matmuls."""
        nbank = 2 if fine else 4
        for ii in range(2):            # d_out halves
            wts = []
            for jh in range(FJ // 4):
                w_ = sb.tile([128, 4, 512], F16, name=f"wd_{e}_{jh}_{ii}",
                             tag="wd", bufs=16)
                nc.sync.dma_start(out=w_, in_=wd[e, jh, ii])
                wts.append(w_)
            for sub in range(4 // nbank):
                grp = [ps.tile([128, TC], F32, name=f"dn_ps_{e}_{ii}_{sub}_{i2}",
                               tag="dn", bufs=4) for i2 in range(nbank)]
                for j in range(FJ):
                    for i2 in range(nbank):
                        ic = sub * nbank + i2
                        nc.tensor.matmul(
                            grp[i2],
                            wts[j // 4][:, j % 4, ic * 128:(ic + 1) * 128],
                            h_tiles[j],
                            start=(j == 0), stop=(j == FJ - 1),
                        )
                for i2 in range(nbank):
                    i = ii * 4 + sub * nbank + i2
                    if first:
                        nc.vector.tensor_copy(acc[i], grp[i2])
                    else:
                        nc.vector.tensor_add(out=acc[i], in0=acc[i], in1=grp[i2])
                    if fine:
                        nc.sync.dma_start(out=yt[i], in_=acc[i])

    acc = [sb.tile([128, TC], F32, name=f"acc{i}") for i in range(KD)]
    rep = [None] * E

    # shared expert's up phase first: it only needs x16 + its weights, so the
    # PE starts ~10us earlier than if the router (which gates on all 8 fp32
    # x chunks) came first. The router runs right after, well before rep[] is
    # needed by up(0)'s scale.
    def _lg_all():
        for tt in range(TC // 128):
            router_logits(tt)

    def _tr_all():
        for tt in range(TC // 128):
            router_transpose(tt)

    # combine-weight rows broadcast across partitions via selector matmuls:
    # rep[e][p, t] = cw[t, e] for all p
    def selectors():
        for e in range(E):
            sel = sb.tile([E, 128], F32R, name=f"sel{e}", tag="sel", bufs=2)
            nc.sync.dma_start(out=sel, in_=selc[e])
            r_ps = ps.tile([128, TC], F32, name=f"rep_ps{e}", tag="dn", bufs=4)
            nc.tensor.matmul(r_ps, sel, cwT, start=True, stop=True)
            r_ = sb.tile([128, TC], F32, name=f"rep{e}")
            nc.scalar.copy(r_, r_ps)
            rep[e] = r_

    h_cur = up_phase(ORDER[0], extras={1: _lg_all, 2: _tr_all, 3: selectors},
                     prefetched=first_wts)

    # software-pipelined main loop: up(e_next) is emitted before down(e) so the
    # PE always has independent matmul work while gelu/scale of e_next runs.
    for idx in range(1, E9):
        h_next = up_phase(ORDER[idx])
        down_phase(ORDER[idx - 1], h_cur, first=(idx == 1))
        h_cur = h_next
    down_phase(ORDER[E9 - 1], h_cur, first=False, fine=True)


def _build():
    if "nc" in _CACHE:
        return _CACHE["nc"]
    nc = bacc.Bacc("TRN2", target_bir_lowering=False, debug=False)
    xt = nc.dram_tensor("xt", [KD, 128, TC], F32R, kind="ExternalInput").ap()
    xt16 = nc.dram_tensor("xt16", [KD, 128, TC], F16, kind="ExternalInput").ap()
    wu = nc.dram_tensor("wu", [E9, 4, KD // 4, 128, 4, 512], F16, kind="ExternalInput").ap()
    wd = nc.dram_tensor("wd", [E9, FJ // 4, 2, 128, 4, 512], F16, kind="ExternalInput").ap()
    wg = nc.dram_tensor("wg", [KD, 128, E], F32, kind="ExternalInput").ap()
    selc = nc.dram_tensor("selc", [E, E, 128], F32R, kind="ExternalInput").ap()
    yt = nc.dram_tensor("yt", [KD, 128, TC], F32, kind="ExternalOutput").ap()
    from contextlib import ExitStack
    with tile.TileContext(nc) as tc, ExitStack() as ctx:
        _emit(nc, tc, ctx, (xt, xt16, wu, wd, wg, selc, yt))
    nc.compile()
    _CACHE["nc"] = nc
    return nc


def kernel(x, Wg, Wu, Wd, W1, W2):
    x = np.ascontiguousarray(x, dtype=np.float32)
    Wg = np.ascontiguousarray(Wg, dtype=np.float32)
    Wu_all = np.concatenate([Wu, W1[None]], axis=0).astype(np.float32)   # [9,D,F]
    Wd_all = np.concatenate([Wd, W2[None]], axis=0).astype(np.float32)   # [9,F,D]

    # host-side tiling to DMA-friendly layouts
    # wu_t[e, jj, kk, p, h, ff] = Wu_all[e, (2*kk+h)*128 + p, jj*512 + ff]
    wu_t = np.ascontiguousarray(
        Wu_all.reshape(E9, KD // 4, 4, 128, 4, 512).transpose(0, 4, 1, 3, 2, 5)
        .astype(np.float16))
    # wd_t[e, jh, ii, p, g, dd] = Wd_all[e, (2*jh+g)*128 + p, ii*512 + dd]
    wd_t = np.ascontiguousarray(
        Wd_all.reshape(E9, FJ // 4, 4, 128, 2, 512).transpose(0, 1, 4, 3, 2, 5)
        .astype(np.float16))
    wg_t = np.ascontiguousarray(Wg.reshape(KD, 128, E))
    sel_c = np.zeros((E, E, 128), dtype=np.float32)
    for e in range(E):
        sel_c[e, e, :] = 1.0

    x2 = x.reshape(T, D)
    in_maps = []
    for c in range(NCORES):
        xt_c = np.ascontiguousarray(
            x2[c * TC:(c + 1) * TC].T).reshape(KD, 128, TC)
        in_maps.append({"xt": xt_c, "xt16": xt_c.astype(np.float16),
                        "wu": wu_t, "wd": wd_t, "wg": wg_t, "selc": sel_c})

    nc = _build()
    trace = bool(os.environ.get("MOE_TRACE"))
    res = run_bass_kernel_spmd(
        nc, in_maps, list(range(NCORES)),
        trace=trace, trace_cores=list(range(NCORES)) if trace else None,
    )
    _CACHE["last_result"] = res

    out = np.empty((T, D), dtype=np.float32)
    for c in range(NCORES):
        out[c * TC:(c + 1) * TC] = res.results[c]["yt"].reshape(D, TC).T
    return out.reshape(B, S, D)
